# revision 12
# baseline (speedup 1.0000x reference)
# Tensor-parallel GQA attention kernel for 8 Trainium2 NeuronCores.
#
# Sharding: each core owns 4 query heads + 1 kv head (32 q / 8 kv heads
# total), computes q/k/v projections for its heads, RoPE, causal
# attention, and a partial o_proj (row slice of wo); the host sums the 8
# partial outputs.
#
# Per-core layout: everything is kept "transposed" ([dim, seq]) so the
# contraction dim of every matmul is the partition axis:
#   qT = wq_c.T @ x.T        [256, S]   (scale folded into wq_c)
#   kvT = wkv_c.T @ x.T      [128, S]   (k rows 0:64, v rows 64:128)
#   scoresT[j, i] = kT.T q   [128-block j, 512-chunk i]  (K=64, 2 heads
#                                                         row-packed)
#   causal mask: an extra matmul accumulates -80 * (1 - mask) into the
#   scores psum ((-80 I).T @ cminv), so exp() of masked entries ~ 1e-33
#   attn_T[d, i] = [v|1].T @ exp(scores)   (row 64 = softmax denoms)
#   out_partial[s, :] = attn_T.T-contracted with wo_c rows
# Normalization happens after PV: one batched reciprocal of the 4 heads'
# denominator rows (gathered at 32-aligned partitions), then a K=128
# selector matmul broadcasts them, then an elementwise multiply.
#
# Persistent tensors (qr, ktd, vtmp, vb) are split into two seq halves
# because Tile tracks dependencies per tile: with single tiles, the
# first attention matmul would wait for the *last* projection write.

import sys
from contextlib import ExitStack

for _p in ("/opt/trn_rl_repo", "/root/.axon_site"):
    if _p not in sys.path:
        sys.path.insert(0, _p)

import numpy as np

import concourse.bacc as bacc
import concourse.mybir as mybir
import concourse.tile as tile
from concourse.bass_utils import run_bass_kernel_spmd
from concourse.masks import make_identity

F32 = mybir.dt.float32
EXP = mybir.ActivationFunctionType.Exp

# matmul operand dtype: float32r (2 cyc/row, ~1.5e-4/matmul) or
# float16 (1 cyc/row, ~7e-4/matmul)
MM_DT = mybir.dt.float16
MASK_NEG = 80.0

S = 2048          # sequence length
H = 2048          # hidden size
NH = 32           # query heads
NKV = 8           # kv heads
HD = 64           # head dim
NCORES = 8
HPC = NH // NCORES        # query heads per core = 4
DQ = HPC * HD             # per-core q width = 256
SCALE = HD ** -0.5
P = 128
NB = S // P               # 16 128-blocks along seq
NC4 = S // 512            # 4 512-chunks along seq
KCH = H // P              # 16 contraction chunks
HS = S // 2               # half seq


def _build_module(mm_dt):
    nc = bacc.Bacc(trn_type="TRN2", debug=False)

    xT_d = nc.dram_tensor("xT", [H, S], mm_dt, kind="ExternalInput").ap()
    wq_d = nc.dram_tensor("wq", [P, KCH * DQ], mm_dt, kind="ExternalInput").ap()
    wkv_d = nc.dram_tensor("wkv", [P, KCH * P], mm_dt, kind="ExternalInput").ap()
    wo_d = nc.dram_tensor("wo", [P, 2 * S], mm_dt, kind="ExternalInput").ap()
    cos_d = nc.dram_tensor("cos2", [P, S], F32, kind="ExternalInput").ap()
    sin_d = nc.dram_tensor("sin2", [P, S], F32, kind="ExternalInput").ap()
    # inverted causal masks for the 4 diagonal offsets, and -80*I
    cm_d = nc.dram_tensor("cminv", [P, 4 * 512], mm_dt, kind="ExternalInput").ap()
    ni_d = nc.dram_tensor("negi", [P, P], mm_dt, kind="ExternalInput").ap()
    # selector matrices for the denominator broadcast
    e0_d = nc.dram_tensor("e0", [P, P], mm_dt, kind="ExternalInput").ap()
    e1_d = nc.dram_tensor("e1", [P, P], mm_dt, kind="ExternalInput").ap()
    out_d = nc.dram_tensor("out", [S, H], F32, kind="ExternalOutput").ap()

    with tile.TileContext(nc) as tc, ExitStack() as ctx:
        pers = ctx.enter_context(tc.tile_pool(name="pers", bufs=1))

        wq_sb = pers.tile([P, KCH * DQ], mm_dt, tag="wq_sb", name="wq_sb")
        wkv_sb = pers.tile([P, KCH * P], mm_dt, tag="wkv_sb", name="wkv_sb")
        cos_sb = pers.tile([P, S], F32, tag="cos_sb", name="cos_sb")
        sin_sb = pers.tile([P, S], F32, tag="sin_sb", name="sin_sb")
        wo_sb = pers.tile([P, 2 * S], mm_dt, tag="wo_sb", name="wo_sb")
        cm_sb = pers.tile([P, 4 * 512], mm_dt, tag="cm_sb", name="cm_sb")
        ni_sb = pers.tile([P, P], mm_dt, tag="ni_sb", name="ni_sb")
        e0_sb = pers.tile([P, P], mm_dt, tag="e0_sb", name="e0_sb")
        e1_sb = pers.tile([P, P], mm_dt, tag="e1_sb", name="e1_sb")
        e_sb = [e0_sb, e1_sb]

        ident = pers.tile([P, P], F32, tag="ident", name="ident")
        make_identity(nc, ident[:])
        ones_f32 = pers.tile([P, HD], F32, tag="ones_f32", name="ones_f32")
        nc.vector.memset(ones_f32[:], 1.0)

        # per-half persistent tensors
        qrh = [[pers.tile([P, HS], mm_dt, tag=f"qr{m}_{g}", name=f"qr{m}_{g}")
                for g in range(2)] for m in range(2)]
        ktdh = [pers.tile([P, HS], mm_dt, tag=f"ktd{g}", name=f"ktd{g}")
                for g in range(2)]
        vtmph = [pers.tile([64, HS], F32, tag=f"vtmp{g}", name=f"vtmp{g}")
                 for g in range(2)]
        vbh = [pers.tile([P, 8 * (HD + 1)], mm_dt, tag=f"vb{g}", name=f"vb{g}")
               for g in range(2)]
        attn0 = pers.tile([P, S], mm_dt, tag="attn0", name="attn0")
        attn1 = pers.tile([P, S], mm_dt, tag="attn1", name="attn1")
        attn = [attn0, attn1]
        rsum = pers.tile([P, 512], F32, tag="rsum", name="rsum")
        nc.vector.memset(rsum[:], 1.0)
        rr = pers.tile([P, 512], mm_dt, tag="rr", name="rr")

        for g in range(2):
            vbv = vbh[g][:].rearrange("p (b c) -> p b c", c=HD + 1)
            nc.vector.tensor_copy(vbv[:, :, HD:HD + 1], ones_f32[:, 0:8])

        # ---------------- projections + RoPE ----------------
        # one iteration per seq half: 2KB-contiguous DMA rows and
        # amortized elementwise overhead
        def proj_half(ph, g):
            xp = ph.enter_context(tc.tile_pool(name=f"xp{g}", bufs=6))
            pp = ph.enter_context(tc.tile_pool(name=f"pp{g}", bufs=6, space="PSUM"))
            tp = ph.enter_context(tc.tile_pool(name=f"tp{g}", bufs=2, space="PSUM"))
            rs = ph.enter_context(tc.tile_pool(name=f"rs{g}", bufs=2))

            nw = slice(1024 * g, 1024 * g + 1024)
            psq = [[None, None], [None, None]]
            pskv = [None, None]
            for half in range(2):
                psq[0][half] = pp.tile([P, 512], F32, tag="pp",
                                       name=f"psq0_{half}")
                psq[1][half] = pp.tile([P, 512], F32, tag="pp",
                                       name=f"psq1_{half}")
                pskv[half] = pp.tile([P, 512], F32, tag="pp",
                                     name=f"pskv_{half}")
            for k in range(KCH):
                if g == 0 and k % 4 == 0:
                    kg = k // 4
                    nc.sync.dma_start(
                        wq_sb[:, 1024 * kg:1024 * kg + 1024],
                        wq_d[:, 1024 * kg:1024 * kg + 1024])
                    nc.sync.dma_start(
                        wkv_sb[:, 512 * kg:512 * kg + 512],
                        wkv_d[:, 512 * kg:512 * kg + 512])
                xt = xp.tile([P, 1024], mm_dt, tag="xt", name="xt")
                nc.sync.dma_start(xt[:], xT_d[P * k:P * k + P, nw])
                st = dict(start=(k == 0), stop=(k == KCH - 1))
                for half in range(2):
                    xs = xt[:, 512 * half:512 * half + 512]
                    nc.tensor.matmul(
                        psq[0][half][:], wq_sb[:, k * DQ:k * DQ + P], xs, **st)
                    nc.tensor.matmul(
                        psq[1][half][:], wq_sb[:, k * DQ + P:k * DQ + DQ],
                        xs, **st)
                    nc.tensor.matmul(
                        pskv[half][:], wkv_sb[:, k * P:k * P + P], xs, **st)
                if g == 0 and k == 3:
                    # trig tables needed by the first RoPE below
                    nc.sync.dma_start(cos_sb[:], cos_d[:])
                    nc.sync.dma_start(sin_sb[:], sin_d[:])
                if g == 0 and k == 9:
                    nc.sync.dma_start(cm_sb[:], cm_d[:])
                    nc.sync.dma_start(ni_sb[:], ni_d[:])
                    nc.sync.dma_start(e0_sb[:], e0_d[:])
                    nc.sync.dma_start(e1_sb[:], e1_d[:])
                    nc.sync.dma_start(wo_sb[:], wo_d[:])

            # RoPE on the two q chunks (2 heads per 128-partition chunk)
            for m in range(2):
                qraw = rs.tile([P, 1024], F32, tag="qraw", name="qraw")
                for half in range(2):
                    nc.scalar.copy(
                        qraw[:, 512 * half:512 * half + 512], psq[m][half][:])
                qsw = rs.tile([P, 1024], F32, tag="qsw", name="qsw")
                for b0 in (0, 64):
                    nc.vector.tensor_copy(
                        qsw[b0:b0 + 32, :], qraw[b0 + 32:b0 + 64, :])
                    nc.vector.tensor_copy(
                        qsw[b0 + 32:b0 + 64, :], qraw[b0:b0 + 32, :])
                nc.vector.tensor_mul(qsw[:], qsw[:], sin_sb[:, nw])
                for half in range(2):
                    hs = slice(512 * half, 512 * half + 512)
                    nc.vector.tensor_mul(
                        qrh[m][g][:, hs], psq[m][half][:],
                        cos_sb[:, 1024 * g + 512 * half:
                               1024 * g + 512 * half + 512])
                nc.vector.tensor_add(qrh[m][g][:], qrh[m][g][:], qsw[:])

            # RoPE on k (kv rows 0:64); v (rows 64:128) has no RoPE
            kraw = rs.tile([64, 1024], F32, tag="kraw", name="kraw")
            for half in range(2):
                nc.scalar.copy(
                    kraw[:, 512 * half:512 * half + 512], pskv[half][0:64, :])
            ksw = rs.tile([64, 1024], F32, tag="ksw", name="ksw")
            nc.vector.tensor_copy(ksw[0:32, :], kraw[32:64, :])
            nc.vector.tensor_copy(ksw[32:64, :], kraw[0:32, :])
            nc.vector.tensor_mul(ksw[:], ksw[:], sin_sb[0:64, nw])
            for half in range(2):
                hs = slice(512 * half, 512 * half + 512)
                nc.vector.tensor_mul(
                    ktdh[g][0:64, hs], pskv[half][0:64, :],
                    cos_sb[0:64, 1024 * g + 512 * half:
                           1024 * g + 512 * half + 512])
            nc.vector.tensor_add(ktdh[g][0:64, :], ktdh[g][0:64, :], ksw[:])
            # duplicate k rows for 2-head row-packed score matmuls
            nc.vector.tensor_copy(ktdh[g][64:P, :], ktdh[g][0:64, :])
            # stash vT, transpose into v blocks
            for half in range(2):
                hs = slice(512 * half, 512 * half + 512)
                nc.scalar.copy(vtmph[g][:, hs], pskv[half][64:P, :])
            for b in range(8):
                pt = tp.tile([P, 64], F32, tag="tp", name="pt")
                nc.tensor.transpose(
                    pt[:], vtmph[g][:, P * b:P * b + P], ident[0:64, 0:64])
                nc.scalar.copy(
                    vbh[g][:, (HD + 1) * b:(HD + 1) * b + HD], pt[:])

        # ---------------- attention + o_proj ----------------
        def attn_ic(ph, pools, ic):
            pop, wkp, pxp, otp = pools
            icg, icr = ic // 2, ic % 2
            nJ = 4 * ic + 4
            po = [
                pop.tile([HD + 1, 512], F32, tag="pop", name=f"po{h}")
                for h in range(HPC)
            ]
            for J in range(nJ):
                t = J - 4 * ic
                # diagonal blocks only contribute to queries i >= 128*t
                c0 = 128 * t if t > 0 else 0
                Jg, Jr = J // 8, J % 8
                Js = slice(P * Jr, P * Jr + P)
                vs = slice((HD + 1) * Jr, (HD + 1) * Jr + HD + 1)
                for hp in range(2):          # head pairs (0,1) and (2,3)
                    pxs = []
                    for hh in range(2):
                        h = 2 * hp + hh
                        m, b0 = h // 2, 64 * (h % 2)
                        qs = slice(512 * icr + c0, 512 * icr + 512)
                        ps_s = wkp.tile([P, 512], F32, tag="wk", name="ps_s")
                        nc.tensor.matmul(
                            ps_s[:, c0:], ktdh[Jg][b0:b0 + 64, Js],
                            qrh[m][icg][b0:b0 + 64, qs],
                            start=True, stop=(t < 0))
                        if t >= 0:
                            # accumulate -80 * inverted causal mask
                            nc.tensor.matmul(
                                ps_s[:, c0:], ni_sb[:],
                                cm_sb[:, 512 * t + c0:512 * t + 512],
                                start=False, stop=True)
                        px = pxp.tile([P, 512], mm_dt, tag="pxp", name="px")
                        nc.scalar.activation(px[:, c0:], ps_s[:, c0:], EXP)
                        pxs.append(px)
                    for hh in range(2):
                        h = 2 * hp + hh
                        nc.tensor.matmul(
                            po[h][:, c0:], vbh[Jg][:, vs], pxs[hh][:, c0:],
                            start=(J == 0), stop=(J == nJ - 1))

            # batched softmax denominators: gather the 4 heads' rows at
            # 32-aligned partitions, one reciprocal, then a K=128
            # selector-matmul broadcast and normalize
            for h in range(HPC):
                nc.scalar.copy(
                    rsum[32 * h:32 * h + 1, :], po[h][HD:HD + 1, :])
            with nc.allow_low_precision(reason="softmax reciprocal"):
                nc.vector.reciprocal(rr[:], rsum[:])
            for m in range(2):
                psb = wkp.tile([P, 512], F32, tag="wk", name="psb")
                nc.tensor.matmul(psb[:], e_sb[m][:], rr[:],
                                 start=True, stop=True)
                for hh in range(2):
                    h, b0 = 2 * m + hh, 64 * hh
                    asl = attn[m][b0:b0 + 64, 512 * ic:512 * ic + 512]
                    nc.scalar.copy(asl, po[h][0:HD, :])
                    nc.vector.tensor_mul(asl, asl, psb[b0:b0 + 64, :])

            for sb in range(4 * ic, 4 * ic + 4):
                ss = slice(P * sb, P * sb + P)
                for n4 in range(NC4):
                    ps_o = wkp.tile([P, 512], F32, tag="wk", name="ps_o")
                    nc.tensor.matmul(
                        ps_o[:], attn0[:, ss],
                        wo_sb[:, 512 * n4:512 * n4 + 512],
                        start=True, stop=False)
                    nc.tensor.matmul(
                        ps_o[:], attn1[:, ss],
                        wo_sb[:, S + 512 * n4:S + 512 * n4 + 512],
                        start=False, stop=True)
                    ot = otp.tile([P, 512], F32, tag="otp", name="ot")
                    nc.vector.tensor_copy(ot[:], ps_o[:])
                    nc.gpsimd.dma_start(out_d[ss, 512 * n4:512 * n4 + 512], ot[:])

        def attn_pair(ph, ic0):
            pop = ph.enter_context(
                tc.tile_pool(name=f"pop{ic0}", bufs=4, space="PSUM"))
            wkp = ph.enter_context(
                tc.tile_pool(name=f"wkp{ic0}", bufs=4, space="PSUM"))
            pxp = ph.enter_context(tc.tile_pool(name=f"pxp{ic0}", bufs=6))
            otp = ph.enter_context(tc.tile_pool(name=f"otp{ic0}", bufs=3))
            pools = (pop, wkp, pxp, otp)
            attn_ic(ph, pools, ic0)
            attn_ic(ph, pools, ic0 + 1)

        with ExitStack() as ph:
            proj_half(ph, 0)
        with ExitStack() as ph:
            attn_pair(ph, 0)
        with ExitStack() as ph:
            proj_half(ph, 1)
        with ExitStack() as ph:
            attn_pair(ph, 2)

    nc.compile()
    return nc


_NC_CACHE = {}


def _get_module(mm_dt=MM_DT):
    if mm_dt not in _NC_CACHE:
        _NC_CACHE[mm_dt] = _build_module(mm_dt)
    return _NC_CACHE[mm_dt]


def _prep_inputs(x, wq, wk, wv, wo, cos, sin, mm_dt=MM_DT):
    mm_np = mybir.dt.np(mm_dt)
    x = np.asarray(x, dtype=np.float32)
    xT = np.ascontiguousarray(x.reshape(S, H).T.astype(mm_np))

    cosT = np.asarray(cos, dtype=np.float32).T          # [64, S]
    sinT = np.asarray(sin, dtype=np.float32).T          # [64, S]
    sgn = np.where(np.arange(HD) < HD // 2, -1.0, 1.0).astype(np.float32)
    sinT_s = sinT * sgn[:, None]
    cos2 = np.ascontiguousarray(np.tile(cosT, (2, 1)))  # [128, S]
    sin2 = np.ascontiguousarray(np.tile(sinT_s, (2, 1)))

    # inverted causal masks (1 where masked out), diagonal offsets 0..3
    jl = np.arange(P)[:, None]
    il = np.arange(512)[None, :]
    cminv = np.concatenate(
        [(jl + P * t > il).astype(np.float32) for t in range(4)], axis=1)
    cminv = np.ascontiguousarray(cminv).astype(mm_np)
    negi = (-MASK_NEG * np.eye(P, dtype=np.float32)).astype(mm_np)

    # selector matrices: psb_m rows 0:64 get the reciprocal row of head
    # 2m (partition 64m), rows 64:128 head 2m+1 (partition 64m+32)
    e0 = np.zeros((P, P), dtype=np.float32)
    e1 = np.zeros((P, P), dtype=np.float32)
    e0[0, 0:64] = 1.0
    e0[32, 64:128] = 1.0
    e1[64, 0:64] = 1.0
    e1[96, 64:128] = 1.0
    e0 = e0.astype(mm_np)
    e1 = e1.astype(mm_np)

    def chunk_kxm(w):
        # [H, M] -> [128, KCH*M] with k-chunk-major free layout
        m = w.shape[1]
        return np.ascontiguousarray(
            w.reshape(KCH, P, m).transpose(1, 0, 2).reshape(P, KCH * m).astype(mm_np))

    wq = np.asarray(wq, dtype=np.float32)
    wk = np.asarray(wk, dtype=np.float32)
    wv = np.asarray(wv, dtype=np.float32)
    wo = np.asarray(wo, dtype=np.float32)

    in_maps = []
    for c in range(NCORES):
        wq_c = wq[:, DQ * c:DQ * c + DQ] * SCALE
        wkv_c = np.concatenate(
            [wk[:, HD * c:HD * c + HD], wv[:, HD * c:HD * c + HD]], axis=1)
        wo_c = wo[DQ * c:DQ * c + DQ, :]
        wo_l = np.ascontiguousarray(
            wo_c.reshape(2, P, H).transpose(1, 0, 2).reshape(P, 2 * H).astype(mm_np))
        in_maps.append({
            "xT": xT,
            "wq": chunk_kxm(wq_c),
            "wkv": chunk_kxm(wkv_c),
            "wo": wo_l,
            "cos2": cos2,
            "sin2": sin2,
            "cminv": cminv,
            "negi": negi,
            "e0": e0,
            "e1": e1,
        })
    return in_maps


def run(inputs, trace=False, trace_kwargs=None, mm_dt=MM_DT):
    """Execute on 8 cores; returns (full_output, BassKernelResults)."""
    nc = _get_module(mm_dt)
    in_maps = _prep_inputs(
        inputs["x"], inputs["wq"], inputs["wk"], inputs["wv"],
        inputs["wo"], inputs["cos"], inputs["sin"], mm_dt=mm_dt)
    kwargs = {}
    if trace:
        kwargs = dict(trace=True, **(trace_kwargs or {}))
    res = run_bass_kernel_spmd(nc, in_maps, core_ids=list(range(NCORES)), **kwargs)
    acc = np.zeros((S, H), dtype=np.float64)
    for c in range(NCORES):
        acc += res.results[c]["out"]
    out = acc.astype(np.float32).reshape(1, S, H)
    return out, res


def kernel(**inputs):
    out, _ = run(inputs, trace=False)
    return out


# revision 13
# speedup vs baseline: 1.3071x; 1.3071x over previous
# Tensor-parallel GQA attention kernel for 8 Trainium2 NeuronCores.
#
# Sharding: each core owns 4 query heads + 1 kv head (32 q / 8 kv heads
# total), computes q/k/v projections for its heads, RoPE, causal
# attention, and a partial o_proj (row slice of wo); the host sums the 8
# partial outputs.
#
# Per-core layout: everything is kept "transposed" ([dim, seq]) so the
# contraction dim of every matmul is the partition axis:
#   qT = wq_c.T @ x.T        [256, S]   (scale folded into wq_c)
#   kvT = wkv_c.T @ x.T      [128, S]   (k rows 0:64, v rows 64:128)
#   scoresT[j, i] = kT.T q   [128-block j, 512-chunk i]  (K=64, 2 heads
#                                                         row-packed)
#   causal mask: an extra matmul accumulates -80 * (1 - mask) into the
#   scores psum ((-80 I).T @ cminv), so exp() of masked entries ~ 1e-33
#   attn_T[d, i] = [v|1].T @ exp(scores)   (row 64 = softmax denoms)
#   out_partial[s, :] = attn_T.T-contracted with wo_c rows
# Normalization happens after PV: one batched reciprocal of the 4 heads'
# denominator rows (gathered at 32-aligned partitions), then a K=128
# selector matmul broadcasts them, then an elementwise multiply.
#
# Scheduling notes:
#  - persistent tensors (qr, ktd, vtmp, vb) are split into two seq
#    halves because Tile tracks dependencies per tile: with single
#    tiles the first attention matmul would wait for the last
#    projection write.
#  - one PSUM pool set serves both phases (projection accumulators
#    share slots with attention psums) so there is no pool-transition
#    barrier.
#  - V is transposed with the DVE 32x32 stream transpose (no PE, no
#    PSUM).

import sys
from contextlib import ExitStack

for _p in ("/opt/trn_rl_repo", "/root/.axon_site"):
    if _p not in sys.path:
        sys.path.insert(0, _p)

import numpy as np

import concourse.bacc as bacc
import concourse.mybir as mybir
import concourse.tile as tile
from concourse.bass_utils import run_bass_kernel_spmd

F32 = mybir.dt.float32
EXP = mybir.ActivationFunctionType.Exp

# matmul operand dtype: float32r (2 cyc/row, ~1.5e-4/matmul) or
# float16 (1 cyc/row, ~7e-4/matmul)
MM_DT = mybir.dt.float16
MASK_NEG = 80.0

S = 2048          # sequence length
H = 2048          # hidden size
NH = 32           # query heads
NKV = 8           # kv heads
HD = 64           # head dim
NCORES = 8
HPC = NH // NCORES        # query heads per core = 4
DQ = HPC * HD             # per-core q width = 256
SCALE = HD ** -0.5
P = 128
NB = S // P               # 16 128-blocks along seq
NC4 = S // 512            # 4 512-chunks along seq
KCH = H // P              # 16 contraction chunks
HS = S // 2               # half seq


def _build_module(mm_dt):
    nc = bacc.Bacc(trn_type="TRN2", debug=False)

    xT_d = nc.dram_tensor("xT", [H, S], mm_dt, kind="ExternalInput").ap()
    wq_d = nc.dram_tensor("wq", [P, KCH * DQ], mm_dt, kind="ExternalInput").ap()
    wkv_d = nc.dram_tensor("wkv", [P, KCH * P], mm_dt, kind="ExternalInput").ap()
    wo_d = nc.dram_tensor("wo", [P, 2 * S], mm_dt, kind="ExternalInput").ap()
    cos_d = nc.dram_tensor("cos2", [P, S], F32, kind="ExternalInput").ap()
    sin_d = nc.dram_tensor("sin2", [P, S], F32, kind="ExternalInput").ap()
    # inverted causal masks for the 4 diagonal offsets, and -80*I
    cm_d = nc.dram_tensor("cminv", [P, 4 * 512], mm_dt, kind="ExternalInput").ap()
    ni_d = nc.dram_tensor("negi", [P, P], mm_dt, kind="ExternalInput").ap()
    # selector matrices for the denominator broadcast
    e0_d = nc.dram_tensor("e0", [P, P], mm_dt, kind="ExternalInput").ap()
    e1_d = nc.dram_tensor("e1", [P, P], mm_dt, kind="ExternalInput").ap()
    out_d = nc.dram_tensor("out", [S, H], F32, kind="ExternalOutput").ap()

    with tile.TileContext(nc) as tc, ExitStack() as ctx:
        pers = ctx.enter_context(tc.tile_pool(name="pers", bufs=1))

        wq_sb = pers.tile([P, KCH * DQ], mm_dt, tag="wq_sb", name="wq_sb")
        wkv_sb = pers.tile([P, KCH * P], mm_dt, tag="wkv_sb", name="wkv_sb")
        cos_sb = pers.tile([P, S], F32, tag="cos_sb", name="cos_sb")
        sin_sb = pers.tile([P, S], F32, tag="sin_sb", name="sin_sb")
        wo_sb = pers.tile([P, 2 * S], mm_dt, tag="wo_sb", name="wo_sb")
        cm_sb = pers.tile([P, 4 * 512], mm_dt, tag="cm_sb", name="cm_sb")
        ni_sb = pers.tile([P, P], mm_dt, tag="ni_sb", name="ni_sb")
        e0_sb = pers.tile([P, P], mm_dt, tag="e0_sb", name="e0_sb")
        e1_sb = pers.tile([P, P], mm_dt, tag="e1_sb", name="e1_sb")
        e_sb = [e0_sb, e1_sb]

        ones16 = pers.tile([P, NB], mm_dt, tag="ones16", name="ones16")
        nc.vector.memset(ones16[:], 1.0)

        # per-half persistent tensors
        qrh = [[pers.tile([P, HS], mm_dt, tag=f"qr{m}_{g}", name=f"qr{m}_{g}")
                for g in range(2)] for m in range(2)]
        ktdh = [pers.tile([P, HS], mm_dt, tag=f"ktd{g}", name=f"ktd{g}")
                for g in range(2)]
        vtmph = [pers.tile([64, HS], mm_dt, tag=f"vtmp{g}", name=f"vtmp{g}")
                 for g in range(2)]
        vbh = [pers.tile([P, 8 * (HD + 1)], mm_dt, tag=f"vb{g}", name=f"vb{g}")
               for g in range(2)]
        attn0 = pers.tile([P, S], mm_dt, tag="attn0", name="attn0")
        attn1 = pers.tile([P, S], mm_dt, tag="attn1", name="attn1")
        attn = [attn0, attn1]
        rsum = pers.tile([P, 512], F32, tag="rsum", name="rsum")
        nc.vector.memset(rsum[:], 1.0)
        rr = pers.tile([P, 512], mm_dt, tag="rr", name="rr")

        for g in range(2):
            vbv = vbh[g][:].rearrange("p (b c) -> p b c", c=HD + 1)
            nc.vector.tensor_copy(vbv[:, :, HD:HD + 1], ones16[:, 0:8])

        # shared pools: one PSUM set for both phases
        xp = ctx.enter_context(tc.tile_pool(name="xp", bufs=6))
        rs = ctx.enter_context(tc.tile_pool(name="rs", bufs=2))
        pop = ctx.enter_context(tc.tile_pool(name="pop", bufs=4, space="PSUM"))
        wkp = ctx.enter_context(tc.tile_pool(name="wkp", bufs=4, space="PSUM"))
        pxp = ctx.enter_context(tc.tile_pool(name="pxp", bufs=10))
        otp = ctx.enter_context(tc.tile_pool(name="otp", bufs=4))

        # ---------------- projections + RoPE ----------------
        def proj_half(g):
            nw = slice(1024 * g, 1024 * g + 1024)
            psq = [[None, None], [None, None]]
            pskv = [None, None]
            for half in range(2):
                psq[0][half] = pop.tile([P, 512], F32, tag="pop",
                                        name=f"psq0_{half}")
                psq[1][half] = pop.tile([P, 512], F32, tag="pop",
                                        name=f"psq1_{half}")
                pskv[half] = wkp.tile([P, 512], F32, tag="wk",
                                      name=f"pskv_{half}")
            for k in range(KCH):
                if g == 0 and k % 4 == 0:
                    kg = k // 4
                    nc.sync.dma_start(
                        wq_sb[:, 1024 * kg:1024 * kg + 1024],
                        wq_d[:, 1024 * kg:1024 * kg + 1024])
                    nc.sync.dma_start(
                        wkv_sb[:, 512 * kg:512 * kg + 512],
                        wkv_d[:, 512 * kg:512 * kg + 512])
                xt = xp.tile([P, 1024], mm_dt, tag="xt", name="xt")
                nc.sync.dma_start(xt[:], xT_d[P * k:P * k + P, nw])
                st = dict(start=(k == 0), stop=(k == KCH - 1))
                for half in range(2):
                    xs = xt[:, 512 * half:512 * half + 512]
                    nc.tensor.matmul(
                        psq[0][half][:], wq_sb[:, k * DQ:k * DQ + P], xs, **st)
                    nc.tensor.matmul(
                        psq[1][half][:], wq_sb[:, k * DQ + P:k * DQ + DQ],
                        xs, **st)
                    nc.tensor.matmul(
                        pskv[half][:], wkv_sb[:, k * P:k * P + P], xs, **st)
                if g == 0 and k == 3:
                    # trig tables needed by the first RoPE below
                    nc.sync.dma_start(cos_sb[:], cos_d[:])
                    nc.sync.dma_start(sin_sb[:], sin_d[:])
                if g == 0 and k == 9:
                    nc.sync.dma_start(cm_sb[:], cm_d[:])
                    nc.sync.dma_start(ni_sb[:], ni_d[:])
                    nc.sync.dma_start(e0_sb[:], e0_d[:])
                    nc.sync.dma_start(e1_sb[:], e1_d[:])
                    nc.sync.dma_start(wo_sb[:], wo_d[:])

            # RoPE on the two q chunks (2 heads per 128-partition chunk)
            for m in range(2):
                qraw = rs.tile([P, 1024], F32, tag="qraw", name="qraw")
                for half in range(2):
                    nc.scalar.copy(
                        qraw[:, 512 * half:512 * half + 512], psq[m][half][:])
                qsw = rs.tile([P, 1024], F32, tag="qsw", name="qsw")
                for b0 in (0, 64):
                    nc.vector.tensor_copy(
                        qsw[b0:b0 + 32, :], qraw[b0 + 32:b0 + 64, :])
                    nc.vector.tensor_copy(
                        qsw[b0 + 32:b0 + 64, :], qraw[b0:b0 + 32, :])
                nc.vector.tensor_mul(qsw[:], qsw[:], sin_sb[:, nw])
                for half in range(2):
                    hs = slice(512 * half, 512 * half + 512)
                    nc.vector.tensor_mul(
                        qrh[m][g][:, hs], psq[m][half][:],
                        cos_sb[:, 1024 * g + 512 * half:
                               1024 * g + 512 * half + 512])
                nc.vector.tensor_add(qrh[m][g][:], qrh[m][g][:], qsw[:])

            # RoPE on k (kv rows 0:64); v (rows 64:128) has no RoPE
            kraw = rs.tile([64, 1024], F32, tag="kraw", name="kraw")
            for half in range(2):
                nc.scalar.copy(
                    kraw[:, 512 * half:512 * half + 512], pskv[half][0:64, :])
            ksw = rs.tile([64, 1024], F32, tag="ksw", name="ksw")
            nc.vector.tensor_copy(ksw[0:32, :], kraw[32:64, :])
            nc.vector.tensor_copy(ksw[32:64, :], kraw[0:32, :])
            nc.vector.tensor_mul(ksw[:], ksw[:], sin_sb[0:64, nw])
            for half in range(2):
                hs = slice(512 * half, 512 * half + 512)
                nc.vector.tensor_mul(
                    ktdh[g][0:64, hs], pskv[half][0:64, :],
                    cos_sb[0:64, 1024 * g + 512 * half:
                           1024 * g + 512 * half + 512])
            nc.vector.tensor_add(ktdh[g][0:64, :], ktdh[g][0:64, :], ksw[:])
            # duplicate k rows for 2-head row-packed score matmuls
            nc.vector.tensor_copy(ktdh[g][64:P, :], ktdh[g][0:64, :])
            # stash vT (fp16) and stream-transpose into v blocks (DVE)
            for half in range(2):
                hs = slice(512 * half, 512 * half + 512)
                nc.scalar.copy(vtmph[g][:, hs], pskv[half][64:P, :])
            vbv = vbh[g][:].rearrange("p (b c) -> p b c", c=HD + 1)
            vtv = vtmph[g][:].rearrange("p (b c) -> p b c", c=P)
            for a in range(4):
                for b in range(2):
                    nc.vector.transpose(
                        vbv[32 * a:32 * a + 32, :, 32 * b:32 * b + 32],
                        vtv[32 * b:32 * b + 32, :, 32 * a:32 * a + 32])

        # ---------------- attention + o_proj ----------------
        def attn_ic(ic):
            icg, icr = ic // 2, ic % 2
            nJ = 4 * ic + 4
            po = [
                pop.tile([HD + 1, 512], F32, tag="pop", name=f"po{h}")
                for h in range(HPC)
            ]
            for J in range(nJ):
                t = J - 4 * ic
                # diagonal blocks only contribute to queries i >= 128*t
                c0 = 128 * t if t > 0 else 0
                Jg, Jr = J // 8, J % 8
                Js = slice(P * Jr, P * Jr + P)
                vs = slice((HD + 1) * Jr, (HD + 1) * Jr + HD + 1)
                for hp in range(2):          # head pairs (0,1) and (2,3)
                    pxs = []
                    for hh in range(2):
                        h = 2 * hp + hh
                        m, b0 = h // 2, 64 * (h % 2)
                        qs = slice(512 * icr + c0, 512 * icr + 512)
                        ps_s = wkp.tile([P, 512], F32, tag="wk", name="ps_s")
                        nc.tensor.matmul(
                            ps_s[:, c0:], ktdh[Jg][b0:b0 + 64, Js],
                            qrh[m][icg][b0:b0 + 64, qs],
                            start=True, stop=(t < 0))
                        if t >= 0:
                            # accumulate -80 * inverted causal mask
                            nc.tensor.matmul(
                                ps_s[:, c0:], ni_sb[:],
                                cm_sb[:, 512 * t + c0:512 * t + 512],
                                start=False, stop=True)
                        px = pxp.tile([P, 512], mm_dt, tag="pxp", name="px")
                        nc.scalar.activation(px[:, c0:], ps_s[:, c0:], EXP)
                        pxs.append(px)
                    for hh in range(2):
                        h = 2 * hp + hh
                        nc.tensor.matmul(
                            po[h][:, c0:], vbh[Jg][:, vs], pxs[hh][:, c0:],
                            start=(J == 0), stop=(J == nJ - 1))

            # batched softmax denominators: gather the 4 heads' rows at
            # 32-aligned partitions, one reciprocal, then a K=128
            # selector-matmul broadcast and normalize
            for h in range(HPC):
                nc.scalar.copy(
                    rsum[32 * h:32 * h + 1, :], po[h][HD:HD + 1, :])
            with nc.allow_low_precision(reason="softmax reciprocal"):
                nc.vector.reciprocal(rr[:], rsum[:])
            for m in range(2):
                psb = wkp.tile([P, 512], F32, tag="wk", name="psb")
                nc.tensor.matmul(psb[:], e_sb[m][:], rr[:],
                                 start=True, stop=True)
                for hh in range(2):
                    h, b0 = 2 * m + hh, 64 * hh
                    asl = attn[m][b0:b0 + 64, 512 * ic:512 * ic + 512]
                    nc.scalar.copy(asl, po[h][0:HD, :])
                    nc.vector.tensor_mul(asl, asl, psb[b0:b0 + 64, :])

            for sb in range(4 * ic, 4 * ic + 4):
                ss = slice(P * sb, P * sb + P)
                for n4 in range(NC4):
                    ps_o = wkp.tile([P, 512], F32, tag="wk", name="ps_o")
                    nc.tensor.matmul(
                        ps_o[:], attn0[:, ss],
                        wo_sb[:, 512 * n4:512 * n4 + 512],
                        start=True, stop=False)
                    nc.tensor.matmul(
                        ps_o[:], attn1[:, ss],
                        wo_sb[:, S + 512 * n4:S + 512 * n4 + 512],
                        start=False, stop=True)
                    ot = otp.tile([P, 512], F32, tag="otp", name="ot")
                    nc.vector.tensor_copy(ot[:], ps_o[:])
                    nc.sync.dma_start(out_d[ss, 512 * n4:512 * n4 + 512], ot[:])

        proj_half(0)
        proj_half(1)
        for ic in range(NC4):
            attn_ic(ic)

    nc.compile()
    return nc


_NC_CACHE = {}


def _get_module(mm_dt=MM_DT):
    if mm_dt not in _NC_CACHE:
        _NC_CACHE[mm_dt] = _build_module(mm_dt)
    return _NC_CACHE[mm_dt]


def _prep_inputs(x, wq, wk, wv, wo, cos, sin, mm_dt=MM_DT):
    mm_np = mybir.dt.np(mm_dt)
    x = np.asarray(x, dtype=np.float32)
    xT = np.ascontiguousarray(x.reshape(S, H).T.astype(mm_np))

    cosT = np.asarray(cos, dtype=np.float32).T          # [64, S]
    sinT = np.asarray(sin, dtype=np.float32).T          # [64, S]
    sgn = np.where(np.arange(HD) < HD // 2, -1.0, 1.0).astype(np.float32)
    sinT_s = sinT * sgn[:, None]
    cos2 = np.ascontiguousarray(np.tile(cosT, (2, 1)))  # [128, S]
    sin2 = np.ascontiguousarray(np.tile(sinT_s, (2, 1)))

    # inverted causal masks (1 where masked out), diagonal offsets 0..3
    jl = np.arange(P)[:, None]
    il = np.arange(512)[None, :]
    cminv = np.concatenate(
        [(jl + P * t > il).astype(np.float32) for t in range(4)], axis=1)
    cminv = np.ascontiguousarray(cminv).astype(mm_np)
    negi = (-MASK_NEG * np.eye(P, dtype=np.float32)).astype(mm_np)

    # selector matrices: psb_m rows 0:64 get the reciprocal row of head
    # 2m (partition 64m), rows 64:128 head 2m+1 (partition 64m+32)
    e0 = np.zeros((P, P), dtype=np.float32)
    e1 = np.zeros((P, P), dtype=np.float32)
    e0[0, 0:64] = 1.0
    e0[32, 64:128] = 1.0
    e1[64, 0:64] = 1.0
    e1[96, 64:128] = 1.0
    e0 = e0.astype(mm_np)
    e1 = e1.astype(mm_np)

    def chunk_kxm(w):
        # [H, M] -> [128, KCH*M] with k-chunk-major free layout
        m = w.shape[1]
        return np.ascontiguousarray(
            w.reshape(KCH, P, m).transpose(1, 0, 2).reshape(P, KCH * m).astype(mm_np))

    wq = np.asarray(wq, dtype=np.float32)
    wk = np.asarray(wk, dtype=np.float32)
    wv = np.asarray(wv, dtype=np.float32)
    wo = np.asarray(wo, dtype=np.float32)

    in_maps = []
    for c in range(NCORES):
        wq_c = wq[:, DQ * c:DQ * c + DQ] * SCALE
        wkv_c = np.concatenate(
            [wk[:, HD * c:HD * c + HD], wv[:, HD * c:HD * c + HD]], axis=1)
        wo_c = wo[DQ * c:DQ * c + DQ, :]
        wo_l = np.ascontiguousarray(
            wo_c.reshape(2, P, H).transpose(1, 0, 2).reshape(P, 2 * H).astype(mm_np))
        in_maps.append({
            "xT": xT,
            "wq": chunk_kxm(wq_c),
            "wkv": chunk_kxm(wkv_c),
            "wo": wo_l,
            "cos2": cos2,
            "sin2": sin2,
            "cminv": cminv,
            "negi": negi,
            "e0": e0,
            "e1": e1,
        })
    return in_maps


def run(inputs, trace=False, trace_kwargs=None, mm_dt=MM_DT):
    """Execute on 8 cores; returns (full_output, BassKernelResults)."""
    nc = _get_module(mm_dt)
    in_maps = _prep_inputs(
        inputs["x"], inputs["wq"], inputs["wk"], inputs["wv"],
        inputs["wo"], inputs["cos"], inputs["sin"], mm_dt=mm_dt)
    kwargs = {}
    if trace:
        kwargs = dict(trace=True, **(trace_kwargs or {}))
    res = run_bass_kernel_spmd(nc, in_maps, core_ids=list(range(NCORES)), **kwargs)
    acc = np.zeros((S, H), dtype=np.float64)
    for c in range(NCORES):
        acc += res.results[c]["out"]
    out = acc.astype(np.float32).reshape(1, S, H)
    return out, res


def kernel(**inputs):
    out, _ = run(inputs, trace=False)
    return out
